# revision 19
# baseline (speedup 1.0000x reference)
"""Trainium2 Bass kernel for the PageRank-propagation problem.

out[i] = (1-C) * sum_j P[i,j] * s[j] / rs[j] + C/n
  P = |Bsym - sim|,  Bsym = triu(B,1) + triu(B,1).T,  rs[j] = sum_k P[j,k]

Sharding: rows split across 8 cores (1024 rows each).  Each core receives its
row block with columns ROTATED by r0 so the compiled SPMD program is identical
on every core: rotated column j'' maps to global column (r0 + j'') mod n.
  j'' in [0, 1024)    : diagonal band (per-element triangular select)
  j'' in [1024, 8192) : off-band; host supplies row-slice / transposed
                        col-slice values directly (layout-only transforms)

Inputs are downcast to bf16 on the host (halves HBM traffic — the kernel is
memory-bound; the final error stays at the 1e-4 level set by the bf16 P
representation).  Host packs each compute tile's operands adjacently so every
SBUF tile needs exactly ONE DMA.

Phase 1 (DMA-bound): per tile, D = X - sim on DVE (bf16 2x mode; diagonal
band tiles assembled with gpsimd affine_selects), then P = |D| on ACT (Abs)
into an SBUF-resident bf16 buffer with the row-sum accumulated for free.
AllGather of the per-core row sums (4 KiB).  Phase 2: t = s * recip(rs)
(f32), rotated via dynamic-offset DMAs from a duplicated DRAM copy, broadcast
across partitions with K=1 PE matmuls, downcast to a resident bf16 t-row by
ACT, then ONE fused multiply+row-sum (scalar_tensor_tensor, standard ISA) per
128-row subblock on DVE over the whole 8192-wide row.
"""

import sys

sys.path.insert(0, "/opt/trn_rl_repo")

import numpy as np

N = 8192
NCORES = 8
NB = N // NCORES          # rows per core (1024)
SB = NB // 128            # 128-row subblocks per core (8)
BAND = NB                 # rotated diagonal band width
W = 1024                  # wide streaming chunk
NONBAND = N - BAND        # 7168
NW = NONBAND // W         # 7 non-band chunks per subblock
PW = N // 512             # 16 t-broadcast chunks (matmul free-dim limit 512)
RS_SLOTS = SB + NW        # rs partial slots per subblock (15)
BPW = 2 * BAND + 128      # bandpack row width (2176)
C = 0.15

_built = {}


def _band_off(ri, cj):
    """Column offset of block (ri, cj) inside the bandpack row."""
    return 2 * 128 * cj + (128 if cj > ri else 0)


def _build():
    if "nc" in _built:
        return _built["nc"]
    import concourse.bass as bass
    import concourse.bacc as bacc
    import concourse.tile as tile
    from concourse import mybir

    dt = mybir.dt
    Alu = mybir.AluOpType
    Act = mybir.ActivationFunctionType

    nc = bacc.Bacc(
        "TRN2", target_bir_lowering=False, debug=False, enable_asserts=False,
        num_devices=NCORES,
    )

    # bsi[i, ci, 0, :] = off-band Bsym chunk; bsi[i, ci, 1, :] = sim chunk
    BSI = nc.dram_tensor("bsi", [NB, NW, 2, W], dt.bfloat16, kind="ExternalInput")
    # per (ri, cj) block: [src|sim] (256 cols) or [bu|bl|sim] on the diagonal
    BP = nc.dram_tensor("bp", [NB, BPW], dt.bfloat16, kind="ExternalInput")
    SV = nc.dram_tensor("sv", [N], dt.float32, kind="ExternalInput")
    OUT = nc.dram_tensor("out", [NB], dt.float32, kind="ExternalOutput")

    with tile.TileContext(nc, num_cores=NCORES) as tc:
        import contextlib

        with contextlib.ExitStack() as ctx:
            constp = ctx.enter_context(tc.tile_pool(name="constp", bufs=1))
            pp = ctx.enter_context(tc.tile_pool(name="pp", bufs=1))
            statp = ctx.enter_context(tc.tile_pool(name="statp", bufs=1))
            psump = ctx.enter_context(
                tc.tile_pool(name="psump", bufs=4, space="PSUM")
            )
            dramp = ctx.enter_context(
                tc.tile_pool(name="dramp", bufs=1, space="DRAM")
            )

            # ---- persistent tiles ----
            P_sb = pp.tile([128, SB * N], dt.bfloat16)      # resident |D|
            ones_t = constp.tile([1, 128], dt.float32)
            rs_part = statp.tile([128, SB * RS_SLOTS], dt.float32)
            rs_sb = statp.tile([128, SB], dt.float32)
            y_sb = statp.tile([128, SB], dt.float32)
            o_sb = statp.tile([128, SB], dt.float32)
            s_small = statp.tile([128, N // 128], dt.float32)
            rs_small = statp.tile([128, N // 128], dt.float32)
            rcp_small = statp.tile([128, N // 128], dt.float32)
            t_small = statp.tile([128, N // 128], dt.float32)

            cc_in = dramp.tile([NB], dt.float32)
            cc_out = dramp.tile([N], dt.float32, addr_space="Shared")
            t_dup = dramp.tile([2 * N], dt.float32)

            nc.gpsimd.memset(ones_t[:], 1.0)
            # s in (p, f) layout: global j = p*64 + f
            nc.sync.dma_start(
                out=s_small[:], in_=SV.ap().rearrange("(p f) -> p f", p=128)
            )

            # ---- phase 1: build P (bf16, SBUF) + row-sum partials ----
            with contextlib.ExitStack() as p1:
                bsip = p1.enter_context(tc.tile_pool(name="bsip", bufs=3))
                bpp = p1.enter_context(tc.tile_pool(name="bpp", bufs=2))
                dtmp = p1.enter_context(tc.tile_pool(name="dtmp", bufs=3))
                xdp = p1.enter_context(tc.tile_pool(name="xdp", bufs=2))

                for ri in range(SB):
                    rowslice = slice(ri * 128, (ri + 1) * 128)
                    pbase = ri * N

                    # band: one DMA for the whole 2176-wide packed row
                    bp_t = bpp.tile([128, BPW], dt.bfloat16, tag="bp")
                    nc.sync.dma_start(out=bp_t[:], in_=BP[rowslice, :])
                    for cj in range(SB):
                        off = _band_off(ri, cj)
                        d_t = dtmp.tile([128, 128], dt.bfloat16, tag="dband")
                        if cj == ri:
                            xu = xdp.tile([128, 128], dt.bfloat16, tag="xu")
                            # strict upper from BU: keep where (f - p) > 0
                            nc.gpsimd.affine_select(
                                out=xu[:], in_=bp_t[:, off:off + 128],
                                compare_op=Alu.is_gt,
                                fill=0.0, base=0, channel_multiplier=-1,
                                pattern=[[1, 128]],
                            )
                            xd = xdp.tile([128, 128], dt.bfloat16, tag="xd")
                            # strict lower from BL: keep where (p - f) > 0
                            nc.gpsimd.affine_select(
                                out=xd[:], in_=bp_t[:, off + 128:off + 256],
                                compare_op=Alu.is_gt,
                                fill=0.0, base=0, channel_multiplier=1,
                                pattern=[[-1, 128]],
                            )
                            nc.gpsimd.tensor_tensor(
                                out=xd[:], in0=xd[:], in1=xu[:], op=Alu.add
                            )
                            nc.gpsimd.tensor_tensor(
                                out=d_t[:], in0=xd[:],
                                in1=bp_t[:, off + 256:off + 384],
                                op=Alu.subtract,
                            )
                        else:
                            nc.vector.tensor_tensor(
                                out=d_t[:], in0=bp_t[:, off:off + 128],
                                in1=bp_t[:, off + 128:off + 256],
                                op=Alu.subtract,
                            )
                        slot = ri * RS_SLOTS + cj
                        nc.scalar.activation(
                            out=P_sb[:, pbase + cj * 128: pbase + (cj + 1) * 128],
                            in_=d_t[:], func=Act.Abs,
                            accum_out=rs_part[:, slot: slot + 1],
                        )

                    # non-band: 7 chunks of 1024 columns, one DMA each
                    for ci in range(NW):
                        bsi_t = bsip.tile([128, 2, W], dt.bfloat16, tag="bsi")
                        nc.sync.dma_start(
                            out=bsi_t[:], in_=BSI[rowslice, ci, :, :]
                        )
                        d_t = dtmp.tile([128, W], dt.bfloat16, tag="dwide")
                        nc.vector.tensor_tensor(
                            out=d_t[:], in0=bsi_t[:, 0, :], in1=bsi_t[:, 1, :],
                            op=Alu.subtract,
                        )
                        slot = ri * RS_SLOTS + SB + ci
                        nc.scalar.activation(
                            out=P_sb[:, pbase + BAND + ci * W: pbase + BAND + (ci + 1) * W],
                            in_=d_t[:], func=Act.Abs,
                            accum_out=rs_part[:, slot: slot + 1],
                        )

            # ---- local row sums -> AllGather ----
            nc.vector.tensor_reduce(
                out=rs_sb[:],
                in_=rs_part[:].rearrange("p (a k) -> p a k", a=SB),
                axis=mybir.AxisListType.X, op=Alu.add,
            )
            # cc_in[g] with g = ri*128 + p  <->  rs_sb[p, ri]
            nc.sync.dma_start(
                out=cc_in[:].rearrange("(a p) -> p a", p=128), in_=rs_sb[:]
            )
            nc.gpsimd.collective_compute(
                "AllGather", Alu.bypass,
                replica_groups=[list(range(NCORES))],
                ins=[cc_in[:]], outs=[cc_out[:]],
            )

            # ---- t = s * recip(rs) in global (p, f) layout ----
            nc.sync.dma_start(
                out=rs_small[:], in_=cc_out[:].rearrange("(p f) -> p f", p=128)
            )
            nc.vector.reciprocal(out=rcp_small[:], in_=rs_small[:])
            nc.vector.tensor_tensor(
                out=t_small[:], in0=s_small[:], in1=rcp_small[:], op=Alu.mult
            )
            # duplicated copy for rotated (wrap-around) reads
            nc.sync.dma_start(
                out=t_dup[0:N].rearrange("(p f) -> p f", p=128), in_=t_small[:]
            )
            nc.sync.dma_start(
                out=t_dup[N:2 * N].rearrange("(p f) -> p f", p=128), in_=t_small[:]
            )

            r0v = nc.partition_id() * NB

            # ---- phase 2: y = sum_j P[:, j''] * t_rot[j''] ----
            with contextlib.ExitStack() as p2:
                tbp = p2.enter_context(tc.tile_pool(name="tbp", bufs=1))
                prodp = p2.enter_context(tc.tile_pool(name="prodp", bufs=1))
                tcip = p2.enter_context(tc.tile_pool(name="tcip", bufs=2))

                tb_full = tbp.tile([128, N], dt.bfloat16)
                prod = prodp.tile([128, N], dt.bfloat16)

                # rotated t -> bf16 broadcast row tb_full:
                # <=8 dynamic-offset DMAs (register budget), K=1 matmul
                # broadcast to PSUM, ACT downcast-copy to SBUF
                for cg in range(8):
                    t_ci = tcip.tile([1, 2 * 512], dt.float32)
                    nc.sync.dma_start(
                        out=t_ci[:],
                        in_=t_dup[bass.ds(r0v + cg * 1024, 1024)][None, :],
                    )
                    for half in range(2):
                        ci = cg * 2 + half
                        ps = psump.tile([128, 512], dt.float32)
                        nc.tensor.matmul(
                            ps[:], ones_t[0:1, :],
                            t_ci[0:1, half * 512:(half + 1) * 512],
                            start=True, stop=True,
                        )
                        nc.scalar.copy(
                            out=tb_full[:, ci * 512:(ci + 1) * 512], in_=ps[:]
                        )

                # one fused multiply+row-sum per 128-row subblock
                # (scalar_tensor_tensor = standard TensorScalarPtr;
                # tensor_tensor_reduce is a custom-DVE op unsupported here)
                for ri in range(SB):
                    nc.vector.scalar_tensor_tensor(
                        out=prod[:],
                        in0=P_sb[:, ri * N:(ri + 1) * N],
                        scalar=1.0, in1=tb_full[:],
                        op0=Alu.bypass, op1=Alu.mult,
                        accum_out=y_sb[:, ri: ri + 1],
                    )

            # out = (1-C) * y + C/n
            nc.scalar.activation(
                out=o_sb[:], in_=y_sb[:],
                func=Act.Copy, bias=float(C / N), scale=float(1.0 - C),
            )
            nc.sync.dma_start(
                out=OUT.ap().rearrange("(a p) -> p a", p=128), in_=o_sb[:]
            )

    nc.finalize()
    _built["nc"] = nc
    return nc


def _shard_inputs(B, sim, s):
    """Layout-only host transforms (slice / transpose / concat / pack),
    plus a bf16 downcast (precision choice of the sharding format)."""
    import ml_dtypes

    bf16 = ml_dtypes.bfloat16
    Bh = B.astype(bf16)
    simh = sim.astype(bf16)
    in_maps = []
    for d in range(NCORES):
        r0, r1 = d * NB, (d + 1) * NB
        # off-band, rotated: global cols [r1..N) then [0..r0)
        bm = np.concatenate(
            [Bh[r0:r1, r1:], np.ascontiguousarray(Bh[:r0, r0:r1].T)], axis=1
        )
        sim_nb = np.concatenate([simh[r0:r1, r1:], simh[r0:r1, :r0]], axis=1)
        bsi = np.stack(
            [bm.reshape(NB, NW, W), sim_nb.reshape(NB, NW, W)], axis=2
        )

        bu = Bh[r0:r1, r0:r1]
        bl = bu.T
        sb = simh[r0:r1, r0:r1]
        rows = []
        for ri in range(SB):
            rs = slice(ri * 128, (ri + 1) * 128)
            pieces = []
            for cj in range(SB):
                cs = slice(cj * 128, (cj + 1) * 128)
                if cj == ri:
                    pieces += [bu[rs, cs], bl[rs, cs], sb[rs, cs]]
                elif cj > ri:
                    pieces += [bu[rs, cs], sb[rs, cs]]
                else:
                    pieces += [bl[rs, cs], sb[rs, cs]]
            rows.append(np.concatenate(pieces, axis=1))
        bp = np.concatenate(rows, axis=0)

        in_maps.append({
            "bsi": np.ascontiguousarray(bsi),
            "bp": np.ascontiguousarray(bp),
            "sv": np.ascontiguousarray(s, dtype=np.float32),
        })
    return in_maps


def kernel(B, similarity_matrix, connectivity_scores, _trace=False, _tmpdir=None):
    from concourse import bass_utils

    B = np.asarray(B, dtype=np.float32)
    sim = np.asarray(similarity_matrix, dtype=np.float32)
    s = np.asarray(connectivity_scores, dtype=np.float32)

    nc = _build()
    in_maps = _shard_inputs(B, sim, s)
    res = bass_utils.run_bass_kernel_spmd(
        nc, in_maps, core_ids=list(range(NCORES)), trace=_trace, tmpdir=_tmpdir
    )
    out = np.concatenate([res.results[d]["out"] for d in range(NCORES)])
    if _trace:
        kernel.last_results = res
    return out


# revision 21
# speedup vs baseline: 1.1756x; 1.1756x over previous
"""Trainium2 Bass kernel for the PageRank-propagation problem.

out[i] = (1-C) * sum_j P[i,j] * s[j] / rs[j] + C/n
  P = |Bsym - sim|,  Bsym = triu(B,1) + triu(B,1).T,  rs[j] = sum_k P[j,k]

Sharding: rows split across 8 cores (1024 rows each).  Each core receives its
row block with columns ROTATED by r0 so the compiled SPMD program is identical
on every core: rotated column j'' maps to global column (r0 + j'') mod n.
  j'' in [0, 1024)    : diagonal band (per-element triangular select)
  j'' in [1024, 8192) : off-band; host supplies row-slice / transposed
                        col-slice values directly (layout-only transforms)

Inputs are downcast to bf16 on the host (halves HBM traffic — the kernel is
memory-bound; the final error stays at the 1e-4 level set by the bf16 P
representation).  Host packs each compute tile's operands adjacently so every
SBUF tile needs exactly ONE DMA.

Phase 1 (DMA-bound): per tile, D = X - sim on DVE (bf16 2x mode; diagonal
band tiles assembled with gpsimd affine_selects), then P = |D| on ACT (Abs)
into an SBUF-resident bf16 buffer with the row-sum accumulated for free.
AllGather of the per-core row sums (4 KiB).  Phase 2: t = s * recip(rs)
(f32), rotated via dynamic-offset DMAs from a duplicated DRAM copy, broadcast
across partitions with K=1 PE matmuls, downcast to a resident bf16 t-row by
ACT, then ONE fused multiply+row-sum (scalar_tensor_tensor, standard ISA) per
128-row subblock on DVE over the whole 8192-wide row.
"""

import sys

sys.path.insert(0, "/opt/trn_rl_repo")

import numpy as np

N = 8192
NCORES = 8
NB = N // NCORES          # rows per core (1024)
SB = NB // 128            # 128-row subblocks per core (8)
BAND = NB                 # rotated diagonal band width
W = 1024                  # wide streaming chunk
NONBAND = N - BAND        # 7168
NW = NONBAND // W         # 7 non-band chunks per subblock
PW = N // 512             # 16 t-broadcast chunks (matmul free-dim limit 512)
RS_SLOTS = SB + NW        # rs partial slots per subblock (15)
BPW = 2 * BAND + 128      # bandpack row width (2176)
C = 0.15

_built = {}


def _band_off(ri, cj):
    """Column offset of block (ri, cj) inside the bandpack row."""
    return 2 * 128 * cj + (128 if cj > ri else 0)


def _P(P_sba, P_sbb, ri):
    """P row-subblock ri lives in half a/b at local offset."""
    half = P_sba if ri < SB // 2 else P_sbb
    base = (ri % (SB // 2)) * N
    return half, base


def _build():
    if "nc" in _built:
        return _built["nc"]
    import concourse.bass as bass
    import concourse.bacc as bacc
    import concourse.tile as tile
    from concourse import mybir

    dt = mybir.dt
    Alu = mybir.AluOpType
    Act = mybir.ActivationFunctionType

    nc = bacc.Bacc(
        "TRN2", target_bir_lowering=False, debug=False, enable_asserts=False,
        num_devices=NCORES,
    )

    # bsi[i, ci, 0, :] = off-band Bsym chunk; bsi[i, ci, 1, :] = sim chunk
    BSI = nc.dram_tensor("bsi", [NB, NW, 2, W], dt.bfloat16, kind="ExternalInput")
    # per (ri, cj) block: [src|sim] (256 cols) or [bu|bl|sim] on the diagonal
    BP = nc.dram_tensor("bp", [NB, BPW], dt.bfloat16, kind="ExternalInput")
    SV = nc.dram_tensor("sv", [N], dt.float32, kind="ExternalInput")
    OUT = nc.dram_tensor("out", [NB], dt.float32, kind="ExternalOutput")

    with tile.TileContext(nc, num_cores=NCORES) as tc:
        import contextlib

        with contextlib.ExitStack() as ctx:
            constp = ctx.enter_context(tc.tile_pool(name="constp", bufs=1))
            pp = ctx.enter_context(tc.tile_pool(name="pp", bufs=1))
            statp = ctx.enter_context(tc.tile_pool(name="statp", bufs=1))
            dramp = ctx.enter_context(
                tc.tile_pool(name="dramp", bufs=1, space="DRAM")
            )

            # ---- persistent tiles ----
            # two halves keep free-dim byte offsets < 64 KiB (the 2x DVE
            # perf mode does not engage on larger AP offsets)
            P_sba = pp.tile([128, SB * N // 2], dt.bfloat16)
            P_sbb = pp.tile([128, SB * N // 2], dt.bfloat16)
            rs_part = statp.tile([128, SB * RS_SLOTS], dt.float32)
            rs_sb = statp.tile([128, SB], dt.float32)
            y_sb = statp.tile([128, SB], dt.float32)
            o_sb = statp.tile([128, SB], dt.float32)
            s_small = statp.tile([128, N // 128], dt.float32)
            rs_small = statp.tile([128, N // 128], dt.float32)
            rcp_small = statp.tile([128, N // 128], dt.float32)
            t_small = statp.tile([128, N // 128], dt.float32)

            cc_in = dramp.tile([NB], dt.float32)
            cc_out = dramp.tile([N], dt.float32, addr_space="Shared")
            t_dup = dramp.tile([2 * N], dt.bfloat16)
            t_bf = statp.tile([128, N // 128], dt.bfloat16)

            # s in (p, f) layout: global j = p*64 + f
            nc.sync.dma_start(
                out=s_small[:], in_=SV.ap().rearrange("(p f) -> p f", p=128)
            )

            # ---- phase 1: build P (bf16, SBUF) + row-sum partials ----
            with contextlib.ExitStack() as p1:
                bsip = p1.enter_context(tc.tile_pool(name="bsip", bufs=5))
                bpp = p1.enter_context(tc.tile_pool(name="bpp", bufs=3))
                dtmp = p1.enter_context(tc.tile_pool(name="dtmp", bufs=3))
                xdp = p1.enter_context(tc.tile_pool(name="xdp", bufs=2))

                for ri in range(SB):
                    rowslice = slice(ri * 128, (ri + 1) * 128)
                    pbase = ri * N

                    # band: one DMA for the whole 2176-wide packed row
                    bp_t = bpp.tile([128, BPW], dt.bfloat16, tag="bp")
                    nc.sync.dma_start(out=bp_t[:], in_=BP[rowslice, :])
                    P_half, pb = _P(P_sba, P_sbb, ri)
                    for cj in range(SB):
                        off = _band_off(ri, cj)
                        d_t = dtmp.tile([128, 128], dt.bfloat16, tag="dband")
                        if cj == ri:
                            xu = xdp.tile([128, 128], dt.bfloat16, tag="xu")
                            # strict upper from BU: keep where (f - p) > 0
                            nc.gpsimd.affine_select(
                                out=xu[:], in_=bp_t[:, off:off + 128],
                                compare_op=Alu.is_gt,
                                fill=0.0, base=0, channel_multiplier=-1,
                                pattern=[[1, 128]],
                            )
                            xd = xdp.tile([128, 128], dt.bfloat16, tag="xd")
                            # strict lower from BL: keep where (p - f) > 0
                            nc.gpsimd.affine_select(
                                out=xd[:], in_=bp_t[:, off + 128:off + 256],
                                compare_op=Alu.is_gt,
                                fill=0.0, base=0, channel_multiplier=1,
                                pattern=[[-1, 128]],
                            )
                            nc.gpsimd.tensor_tensor(
                                out=xd[:], in0=xd[:], in1=xu[:], op=Alu.add
                            )
                            nc.gpsimd.tensor_tensor(
                                out=d_t[:], in0=xd[:],
                                in1=bp_t[:, off + 256:off + 384],
                                op=Alu.subtract,
                            )
                        else:
                            nc.vector.tensor_tensor(
                                out=d_t[:], in0=bp_t[:, off:off + 128],
                                in1=bp_t[:, off + 128:off + 256],
                                op=Alu.subtract,
                            )
                        slot = ri * RS_SLOTS + cj
                        nc.scalar.activation(
                            out=P_half[:, pb + cj * 128: pb + (cj + 1) * 128],
                            in_=d_t[:], func=Act.Abs,
                            accum_out=rs_part[:, slot: slot + 1],
                        )

                    # non-band: 7 chunks of 1024 columns, one DMA each
                    for ci in range(NW):
                        bsi_t = bsip.tile([128, 2, W], dt.bfloat16, tag="bsi")
                        nc.sync.dma_start(
                            out=bsi_t[:], in_=BSI[rowslice, ci, :, :]
                        )
                        d_t = dtmp.tile([128, W], dt.bfloat16, tag="dwide")
                        nc.vector.tensor_tensor(
                            out=d_t[:], in0=bsi_t[:, 0, :], in1=bsi_t[:, 1, :],
                            op=Alu.subtract,
                        )
                        slot = ri * RS_SLOTS + SB + ci
                        nc.scalar.activation(
                            out=P_half[:, pb + BAND + ci * W: pb + BAND + (ci + 1) * W],
                            in_=d_t[:], func=Act.Abs,
                            accum_out=rs_part[:, slot: slot + 1],
                        )

            # ---- local row sums -> AllGather ----
            nc.vector.tensor_reduce(
                out=rs_sb[:],
                in_=rs_part[:].rearrange("p (a k) -> p a k", a=SB),
                axis=mybir.AxisListType.X, op=Alu.add,
            )
            # cc_in[g] with g = ri*128 + p  <->  rs_sb[p, ri]
            nc.sync.dma_start(
                out=cc_in[:].rearrange("(a p) -> p a", p=128), in_=rs_sb[:]
            )
            nc.gpsimd.collective_compute(
                "AllGather", Alu.bypass,
                replica_groups=[list(range(NCORES))],
                ins=[cc_in[:]], outs=[cc_out[:]],
            )

            # ---- t = s * recip(rs) in global (p, f) layout ----
            nc.sync.dma_start(
                out=rs_small[:], in_=cc_out[:].rearrange("(p f) -> p f", p=128)
            )
            nc.vector.reciprocal(out=rcp_small[:], in_=rs_small[:])
            nc.vector.tensor_tensor(
                out=t_small[:], in0=s_small[:], in1=rcp_small[:], op=Alu.mult
            )
            nc.scalar.activation(out=t_bf[:], in_=t_small[:], func=Act.Copy)
            # duplicated copy for rotated (wrap-around) reads
            nc.sync.dma_start(
                out=t_dup[0:N].rearrange("(p f) -> p f", p=128), in_=t_bf[:]
            )
            nc.sync.dma_start(
                out=t_dup[N:2 * N].rearrange("(p f) -> p f", p=128), in_=t_bf[:]
            )

            r0v = nc.partition_id() * NB

            # ---- phase 2: y = sum_j P[:, j''] * t_rot[j''] ----
            with contextlib.ExitStack() as p2:
                tbp = p2.enter_context(tc.tile_pool(name="tbp", bufs=1))
                prodp = p2.enter_context(tc.tile_pool(name="prodp", bufs=1))

                tb_full = tbp.tile([128, N], dt.bfloat16)
                prod = prodp.tile([128, N], dt.bfloat16)

                # rotated t, replicated across partitions by a single
                # 0-stride-partition DMA from the duplicated DRAM copy
                nc.sync.dma_start(
                    out=tb_full[:],
                    in_=t_dup[bass.ds(r0v, N)].partition_broadcast(128),
                )

                # one fused multiply+row-sum per 128-row subblock
                # (scalar_tensor_tensor = standard TensorScalarPtr;
                # tensor_tensor_reduce is a custom-DVE op unsupported here)
                for ri in range(SB):
                    P_half, pb = _P(P_sba, P_sbb, ri)
                    nc.vector.scalar_tensor_tensor(
                        out=prod[:],
                        in0=P_half[:, pb:pb + N],
                        scalar=1.0, in1=tb_full[:],
                        op0=Alu.bypass, op1=Alu.mult,
                        accum_out=y_sb[:, ri: ri + 1],
                    )

            # out = (1-C) * y + C/n
            nc.scalar.activation(
                out=o_sb[:], in_=y_sb[:],
                func=Act.Copy, bias=float(C / N), scale=float(1.0 - C),
            )
            nc.sync.dma_start(
                out=OUT.ap().rearrange("(a p) -> p a", p=128), in_=o_sb[:]
            )

    nc.finalize()
    _built["nc"] = nc
    return nc


def _shard_inputs(B, sim, s):
    """Layout-only host transforms (slice / transpose / concat / pack),
    plus a bf16 downcast (precision choice of the sharding format)."""
    import ml_dtypes

    bf16 = ml_dtypes.bfloat16
    Bh = B.astype(bf16)
    simh = sim.astype(bf16)
    in_maps = []
    for d in range(NCORES):
        r0, r1 = d * NB, (d + 1) * NB
        # off-band, rotated: global cols [r1..N) then [0..r0)
        bm = np.concatenate(
            [Bh[r0:r1, r1:], np.ascontiguousarray(Bh[:r0, r0:r1].T)], axis=1
        )
        sim_nb = np.concatenate([simh[r0:r1, r1:], simh[r0:r1, :r0]], axis=1)
        bsi = np.stack(
            [bm.reshape(NB, NW, W), sim_nb.reshape(NB, NW, W)], axis=2
        )

        bu = Bh[r0:r1, r0:r1]
        bl = bu.T
        sb = simh[r0:r1, r0:r1]
        rows = []
        for ri in range(SB):
            rs = slice(ri * 128, (ri + 1) * 128)
            pieces = []
            for cj in range(SB):
                cs = slice(cj * 128, (cj + 1) * 128)
                if cj == ri:
                    pieces += [bu[rs, cs], bl[rs, cs], sb[rs, cs]]
                elif cj > ri:
                    pieces += [bu[rs, cs], sb[rs, cs]]
                else:
                    pieces += [bl[rs, cs], sb[rs, cs]]
            rows.append(np.concatenate(pieces, axis=1))
        bp = np.concatenate(rows, axis=0)

        in_maps.append({
            "bsi": np.ascontiguousarray(bsi),
            "bp": np.ascontiguousarray(bp),
            "sv": np.ascontiguousarray(s, dtype=np.float32),
        })
    return in_maps


def kernel(B, similarity_matrix, connectivity_scores, _trace=False, _tmpdir=None):
    from concourse import bass_utils

    B = np.asarray(B, dtype=np.float32)
    sim = np.asarray(similarity_matrix, dtype=np.float32)
    s = np.asarray(connectivity_scores, dtype=np.float32)

    nc = _build()
    in_maps = _shard_inputs(B, sim, s)
    res = bass_utils.run_bass_kernel_spmd(
        nc, in_maps, core_ids=list(range(NCORES)), trace=_trace, tmpdir=_tmpdir
    )
    out = np.concatenate([res.results[d]["out"] for d in range(NCORES)])
    if _trace:
        kernel.last_results = res
    return out


# revision 22
# speedup vs baseline: 1.2408x; 1.0555x over previous
"""Trainium2 Bass kernel for the PageRank-propagation problem.

out[i] = (1-C) * sum_j P[i,j] * s[j] / rs[j] + C/n
  P = |Bsym - sim|,  Bsym = triu(B,1) + triu(B,1).T,  rs[j] = sum_k P[j,k]

Sharding: rows split across 8 cores (1024 rows each).  Each core receives its
row block with columns ROTATED by r0 so the compiled SPMD program is identical
on every core: rotated column j'' maps to global column (r0 + j'') mod n.
  j'' in [0, 1024)    : diagonal band (per-element triangular select)
  j'' in [1024, 8192) : off-band; host supplies row-slice / transposed
                        col-slice values directly (layout-only transforms)

Inputs are downcast to bf16 on the host (halves HBM traffic — the kernel is
memory-bound; the final error stays at the 1e-4 level set by the bf16 P
representation).  Host packs each compute tile's operands adjacently so every
SBUF tile needs exactly ONE DMA.

Phase 1 (DMA-bound): per tile, D = X - sim on DVE (bf16 2x mode; diagonal
band tiles assembled with gpsimd affine_selects), then P = |D| on ACT (Abs)
into an SBUF-resident bf16 buffer with the row-sum accumulated for free.
AllGather of the per-core row sums (4 KiB).  Phase 2: t = s * recip(rs)
(f32), rotated via dynamic-offset DMAs from a duplicated DRAM copy, broadcast
across partitions with K=1 PE matmuls, downcast to a resident bf16 t-row by
ACT, then ONE fused multiply+row-sum (scalar_tensor_tensor, standard ISA) per
128-row subblock on DVE over the whole 8192-wide row.
"""

import sys

sys.path.insert(0, "/opt/trn_rl_repo")

import numpy as np

N = 8192
NCORES = 8
NB = N // NCORES          # rows per core (1024)
SB = NB // 128            # 128-row subblocks per core (8)
BAND = NB                 # rotated diagonal band width
W = 1024                  # wide streaming chunk
NONBAND = N - BAND        # 7168
NW = NONBAND // W         # 7 non-band chunks per subblock
PW = N // 512             # 16 t-broadcast chunks (matmul free-dim limit 512)
RS_SLOTS = SB + NW        # rs partial slots per subblock (15)
BPW = 2 * BAND + 128      # bandpack row width (2176)
C = 0.15

_built = {}


def _band_off(ri, cj):
    """Column offset of block (ri, cj) inside the bandpack row."""
    return 2 * 128 * cj + (128 if cj > ri else 0)


def _P(P_sba, P_sbb, ri):
    """P row-subblock ri lives in half a/b at local offset."""
    half = P_sba if ri < SB // 2 else P_sbb
    base = (ri % (SB // 2)) * N
    return half, base


def _build():
    if "nc" in _built:
        return _built["nc"]
    import concourse.bass as bass
    import concourse.bacc as bacc
    import concourse.tile as tile
    from concourse import mybir

    dt = mybir.dt
    Alu = mybir.AluOpType
    Act = mybir.ActivationFunctionType

    nc = bacc.Bacc(
        "TRN2", target_bir_lowering=False, debug=False, enable_asserts=False,
        num_devices=NCORES,
    )

    # bsi[i, ci, 0, :] = off-band Bsym chunk; bsi[i, ci, 1, :] = sim chunk
    BSI = nc.dram_tensor("bsi", [NB, NW, 2, W], dt.bfloat16, kind="ExternalInput")
    # per (ri, cj) block: [src|sim] (256 cols) or [bu|bl|sim] on the diagonal
    BP = nc.dram_tensor("bp", [NB, BPW], dt.bfloat16, kind="ExternalInput")
    SV = nc.dram_tensor("sv", [N], dt.float32, kind="ExternalInput")
    OUT = nc.dram_tensor("out", [NB], dt.float32, kind="ExternalOutput")

    with tile.TileContext(nc, num_cores=NCORES) as tc:
        import contextlib

        with contextlib.ExitStack() as ctx:
            constp = ctx.enter_context(tc.tile_pool(name="constp", bufs=1))
            pp = ctx.enter_context(tc.tile_pool(name="pp", bufs=1))
            statp = ctx.enter_context(tc.tile_pool(name="statp", bufs=1))
            dramp = ctx.enter_context(
                tc.tile_pool(name="dramp", bufs=1, space="DRAM")
            )

            # ---- persistent tiles ----
            # two halves keep free-dim byte offsets < 64 KiB (the 2x DVE
            # perf mode does not engage on larger AP offsets)
            P_sba = pp.tile([128, SB * N // 2], dt.bfloat16)
            P_sbb = pp.tile([128, SB * N // 2], dt.bfloat16)
            rs_part = statp.tile([128, SB * RS_SLOTS], dt.float32)
            rs_sb = statp.tile([128, SB], dt.float32)
            y_sb = statp.tile([128, SB], dt.float32)
            o_sb = statp.tile([128, SB], dt.float32)
            s_small = statp.tile([128, N // 128], dt.float32)
            rs_small = statp.tile([128, N // 128], dt.float32)
            rcp_small = statp.tile([128, N // 128], dt.float32)
            t_small = statp.tile([128, N // 128], dt.float32)

            cc_in = dramp.tile([NB], dt.float32)
            cc_out = dramp.tile([N], dt.float32, addr_space="Shared")
            t_dup = dramp.tile([2 * N], dt.bfloat16)
            t_bf = statp.tile([128, N // 128], dt.bfloat16)

            # s in (p, f) layout: global j = p*64 + f
            nc.sync.dma_start(
                out=s_small[:], in_=SV.ap().rearrange("(p f) -> p f", p=128)
            )

            # ---- phase 1: build P (bf16, SBUF) + row-sum partials ----
            with contextlib.ExitStack() as p1:
                bsip = p1.enter_context(tc.tile_pool(name="bsip", bufs=5))
                bpp = p1.enter_context(tc.tile_pool(name="bpp", bufs=3))
                dtmp = p1.enter_context(tc.tile_pool(name="dtmp", bufs=3))
                xdp = p1.enter_context(tc.tile_pool(name="xdp", bufs=2))

                for ri in range(SB):
                    rowslice = slice(ri * 128, (ri + 1) * 128)
                    pbase = ri * N

                    # band: one DMA for the whole 2176-wide packed row
                    bp_t = bpp.tile([128, BPW], dt.bfloat16, tag="bp")
                    nc.sync.dma_start(out=bp_t[:], in_=BP[rowslice, :])
                    P_half, pb = _P(P_sba, P_sbb, ri)
                    for cj in range(SB):
                        off = _band_off(ri, cj)
                        d_t = dtmp.tile([128, 128], dt.bfloat16, tag="dband")
                        if cj == ri:
                            xu = xdp.tile([128, 128], dt.bfloat16, tag="xu")
                            # strict upper from BU: keep where (f - p) > 0
                            nc.gpsimd.affine_select(
                                out=xu[:], in_=bp_t[:, off:off + 128],
                                compare_op=Alu.is_gt,
                                fill=0.0, base=0, channel_multiplier=-1,
                                pattern=[[1, 128]],
                            )
                            xd = xdp.tile([128, 128], dt.bfloat16, tag="xd")
                            # strict lower from BL: keep where (p - f) > 0
                            nc.gpsimd.affine_select(
                                out=xd[:], in_=bp_t[:, off + 128:off + 256],
                                compare_op=Alu.is_gt,
                                fill=0.0, base=0, channel_multiplier=1,
                                pattern=[[-1, 128]],
                            )
                            nc.gpsimd.tensor_tensor(
                                out=xd[:], in0=xd[:], in1=xu[:], op=Alu.add
                            )
                            nc.gpsimd.tensor_tensor(
                                out=d_t[:], in0=xd[:],
                                in1=bp_t[:, off + 256:off + 384],
                                op=Alu.subtract,
                            )
                        else:
                            nc.vector.tensor_tensor(
                                out=d_t[:], in0=bp_t[:, off:off + 128],
                                in1=bp_t[:, off + 128:off + 256],
                                op=Alu.subtract,
                            )
                        slot = ri * RS_SLOTS + cj
                        nc.scalar.activation(
                            out=P_half[:, pb + cj * 128: pb + (cj + 1) * 128],
                            in_=d_t[:], func=Act.Abs,
                            accum_out=rs_part[:, slot: slot + 1],
                        )

                    # non-band: 7 chunks of 1024 columns, one DMA each
                    for ci in range(NW):
                        bsi_t = bsip.tile([128, 2, W], dt.bfloat16, tag="bsi")
                        nc.sync.dma_start(
                            out=bsi_t[:], in_=BSI[rowslice, ci, :, :]
                        )
                        d_t = dtmp.tile([128, W], dt.bfloat16, tag="dwide")
                        nc.vector.tensor_tensor(
                            out=d_t[:], in0=bsi_t[:, 0, :], in1=bsi_t[:, 1, :],
                            op=Alu.subtract,
                        )
                        slot = ri * RS_SLOTS + SB + ci
                        nc.scalar.activation(
                            out=P_half[:, pb + BAND + ci * W: pb + BAND + (ci + 1) * W],
                            in_=d_t[:], func=Act.Abs,
                            accum_out=rs_part[:, slot: slot + 1],
                        )

            # ---- local row sums -> AllGather ----
            nc.vector.tensor_reduce(
                out=rs_sb[:],
                in_=rs_part[:].rearrange("p (a k) -> p a k", a=SB),
                axis=mybir.AxisListType.X, op=Alu.add,
            )
            # cc_in[g] with g = ri*128 + p  <->  rs_sb[p, ri]
            nc.sync.dma_start(
                out=cc_in[:].rearrange("(a p) -> p a", p=128), in_=rs_sb[:]
            )
            nc.gpsimd.collective_compute(
                "AllGather", Alu.bypass,
                replica_groups=[list(range(NCORES))],
                ins=[cc_in[:]], outs=[cc_out[:]],
            )

            # ---- t = s * recip(rs) in global (p, f) layout ----
            nc.sync.dma_start(
                out=rs_small[:], in_=cc_out[:].rearrange("(p f) -> p f", p=128)
            )
            nc.vector.reciprocal(out=rcp_small[:], in_=rs_small[:])
            nc.vector.tensor_tensor(
                out=t_small[:], in0=s_small[:], in1=rcp_small[:], op=Alu.mult
            )
            nc.scalar.activation(out=t_bf[:], in_=t_small[:], func=Act.Copy)
            # duplicated copy for rotated (wrap-around) reads
            nc.sync.dma_start(
                out=t_dup[0:N].rearrange("(p f) -> p f", p=128), in_=t_bf[:]
            )
            nc.sync.dma_start(
                out=t_dup[N:2 * N].rearrange("(p f) -> p f", p=128), in_=t_bf[:]
            )

            r0v = nc.partition_id() * NB

            # ---- phase 2: y = sum_j P[:, j''] * t_rot[j''] ----
            with contextlib.ExitStack() as p2:
                tbp = p2.enter_context(tc.tile_pool(name="tbp", bufs=1))

                # pad keeps tb_full 128B-aligned in SBUF
                tb_pad = tbp.tile([128, 32], dt.bfloat16)
                tb_full = tbp.tile([128, N], dt.bfloat16)

                # rotated t, replicated across partitions by a single
                # 0-stride-partition DMA from the duplicated DRAM copy
                nc.sync.dma_start(
                    out=tb_full[:],
                    in_=t_dup[bass.ds(r0v, N)].partition_broadcast(128),
                )

                # one fused multiply+row-sum per 128-row subblock
                # (scalar_tensor_tensor = standard TensorScalarPtr;
                # tensor_tensor_reduce is a custom-DVE op unsupported here)
                for ri in range(SB):
                    P_half, pb = _P(P_sba, P_sbb, ri)
                    # in-place: P is dead after its single phase-2 read
                    nc.vector.scalar_tensor_tensor(
                        out=P_half[:, pb:pb + N],
                        in0=P_half[:, pb:pb + N],
                        scalar=1.0, in1=tb_full[:],
                        op0=Alu.bypass, op1=Alu.mult,
                        accum_out=y_sb[:, ri: ri + 1],
                    )

            # out = (1-C) * y + C/n
            nc.scalar.activation(
                out=o_sb[:], in_=y_sb[:],
                func=Act.Copy, bias=float(C / N), scale=float(1.0 - C),
            )
            nc.sync.dma_start(
                out=OUT.ap().rearrange("(a p) -> p a", p=128), in_=o_sb[:]
            )

    nc.finalize()
    _built["nc"] = nc
    return nc


def _shard_inputs(B, sim, s):
    """Layout-only host transforms (slice / transpose / concat / pack),
    plus a bf16 downcast (precision choice of the sharding format)."""
    import ml_dtypes

    bf16 = ml_dtypes.bfloat16
    Bh = B.astype(bf16)
    simh = sim.astype(bf16)
    in_maps = []
    for d in range(NCORES):
        r0, r1 = d * NB, (d + 1) * NB
        # off-band, rotated: global cols [r1..N) then [0..r0)
        bm = np.concatenate(
            [Bh[r0:r1, r1:], np.ascontiguousarray(Bh[:r0, r0:r1].T)], axis=1
        )
        sim_nb = np.concatenate([simh[r0:r1, r1:], simh[r0:r1, :r0]], axis=1)
        bsi = np.stack(
            [bm.reshape(NB, NW, W), sim_nb.reshape(NB, NW, W)], axis=2
        )

        bu = Bh[r0:r1, r0:r1]
        bl = bu.T
        sb = simh[r0:r1, r0:r1]
        rows = []
        for ri in range(SB):
            rs = slice(ri * 128, (ri + 1) * 128)
            pieces = []
            for cj in range(SB):
                cs = slice(cj * 128, (cj + 1) * 128)
                if cj == ri:
                    pieces += [bu[rs, cs], bl[rs, cs], sb[rs, cs]]
                elif cj > ri:
                    pieces += [bu[rs, cs], sb[rs, cs]]
                else:
                    pieces += [bl[rs, cs], sb[rs, cs]]
            rows.append(np.concatenate(pieces, axis=1))
        bp = np.concatenate(rows, axis=0)

        in_maps.append({
            "bsi": np.ascontiguousarray(bsi),
            "bp": np.ascontiguousarray(bp),
            "sv": np.ascontiguousarray(s, dtype=np.float32),
        })
    return in_maps


def kernel(B, similarity_matrix, connectivity_scores, _trace=False, _tmpdir=None):
    from concourse import bass_utils

    B = np.asarray(B, dtype=np.float32)
    sim = np.asarray(similarity_matrix, dtype=np.float32)
    s = np.asarray(connectivity_scores, dtype=np.float32)

    nc = _build()
    in_maps = _shard_inputs(B, sim, s)
    res = bass_utils.run_bass_kernel_spmd(
        nc, in_maps, core_ids=list(range(NCORES)), trace=_trace, tmpdir=_tmpdir
    )
    out = np.concatenate([res.results[d]["out"] for d in range(NCORES)])
    if _trace:
        kernel.last_results = res
    return out
